# revision 14
# baseline (speedup 1.0000x reference)
"""Gaussian splatting renderer on 8 Trainium2 NeuronCores.

v2 — optimized pipeline (baseline was 375us):
  - S[p, n] = -Mahalanobis^2 via a split-fp16 rank-18 matmul (hi/lo
    Dekker split of both operands, coords recentered per core so fp16
    magnitudes stay small).  fp16 streams 1 col/cycle vs ~3 for fp32.
  - ACT converts PSUM fp32 -> SBUF fp16 score array (graded float
    precision is ideal: top alphas have |S| near 0).
  - DVE top-k on fp16 at 2x rate: max8 per 512-chunk, merge with
    max8/match_replace/max8 (ranks 1..16), find_index8 for ids.
  - exp batched into one ACT instruction at the end (no table thrash).
  - per-gaussian features computed on device in [128,16] layout, then
    PE-transposed (no DRAM round trip); gather color table fed
    host-permuted so transposed column order needs no index fixup.
  - dummy early ap_gather pre-loads the gpsimd ucode so its expensive
    drain overlaps the main loop instead of serializing the epilogue.
  - color gather only fetches the K=10 used slots (s-major index
    layout), diagonal-mask + reduce as in baseline but 37% smaller.
"""

import os
import sys

sys.path.insert(0, "/opt/trn_rl_repo")

import numpy as np
from contextlib import ExitStack

from concourse import bacc, bass, mybir, tile
from concourse.bass_utils import run_bass_kernel_spmd

H, W, K = 128, 128, 10
N_G = 2048
N_CORES = 8
ROWS_PER_CORE = H // N_CORES          # 16
T_TILES = ROWS_PER_CORE               # one image row per tile
PPT = 128                             # pixels per tile (one row)
SLOTS = 16                            # padded top-k slots (ranks 1..16)
F32 = mybir.dt.float32
F16 = mybir.dt.float16
U16 = mybir.dt.uint16
I16 = mybir.dt.int16

NEG_SENT = -60000.0                   # fp16-representable sentinel
XC = 63.5                             # x recenter offset


def build_program():
    nc = bacc.Bacc(
        "TRN2",
        target_bir_lowering=False,
        debug=False,
        num_devices=N_CORES,
    )

    means = nc.dram_tensor("g_means", [N_G, 2], F32, kind="ExternalInput")
    rots = nc.dram_tensor("g_rots", [N_G], F32, kind="ExternalInput")
    lss = nc.dram_tensor("g_ls", [N_G, 2], F32, kind="ExternalInput")
    ctab = nc.dram_tensor("g_ctab", [N_G, 3], F32, kind="ExternalInput")
    pfh = nc.dram_tensor("pf_hi", [T_TILES, 6, PPT], F16, kind="ExternalInput")
    pfl = nc.dram_tensor("pf_lo", [T_TILES, 6, PPT], F16, kind="ExternalInput")
    ident = nc.dram_tensor("ident", [128, 128], F16, kind="ExternalInput")
    dmask = nc.dram_tensor("dmask", [128, 768], F32, kind="ExternalInput")
    ycen = nc.dram_tensor("ycen", [128, 1], F32, kind="ExternalInput")
    out = nc.dram_tensor("out", [ROWS_PER_CORE * W, 3], F32, kind="ExternalOutput")

    with tile.TileContext(nc) as tc, ExitStack() as ctx:
        persist = ctx.enter_context(tc.tile_pool(name="persist", bufs=1))
        work = ctx.enter_context(tc.tile_pool(name="work", bufs=3))

        # ---------------- input DMAs (all contiguous, sync queue) -------------
        mjc = persist.tile([128, 32], F32, tag="mjc")      # means (j c) interleaved
        rot = persist.tile([128, 16], F32, tag="rot")
        ljc = persist.tile([128, 32], F32, tag="ljc")      # log scales (j c)
        id16 = persist.tile([128, 128], F16, tag="id16")
        yc = persist.tile([128, 1], F32, tag="yc")
        nc.sync.dma_start(mjc[:], means.ap().rearrange("(p j) c -> p (j c)", p=128))
        nc.sync.dma_start(rot[:], rots.ap().rearrange("(p j) -> p j", p=128))
        nc.sync.dma_start(ljc[:], lss.ap().rearrange("(p j) c -> p (j c)", p=128))
        nc.sync.dma_start(id16[:], ident.ap())
        nc.sync.dma_start(yc[:], ycen.ap())

        # broadcast color table [128, 6144] (column order matches grhs layout;
        # host pre-permutes) and diagonal mask replicated x160
        coltab = persist.tile([128, N_G * 3], F32, tag="coltab")
        nc.sync.dma_start(
            coltab[:],
            ctab.ap().rearrange("n c -> (n c)").unsqueeze(0).broadcast_to([128, N_G * 3]),
        )
        dm10 = persist.tile([128, K * T_TILES * 48], F32, tag="dm10")
        nc.sync.dma_start(
            dm10[:].rearrange("p (u q) -> p u q", q=768),
            dmask.ap().unsqueeze(1).broadcast_to([128, K, 768]),
        )

        # pixel features lhsT [18, 16*128] fp16: rows 0-5 hi, 6-11 lo, 12-17 hi
        plhs = persist.tile([18, T_TILES * PPT], F16, tag="plhs")
        nc.sync.dma_start(
            plhs[0:6].rearrange("f (t j) -> f t j", t=T_TILES),
            pfh.ap().rearrange("t f j -> f t j"),
        )
        nc.sync.dma_start(
            plhs[6:12].rearrange("f (t j) -> f t j", t=T_TILES),
            pfl.ap().rearrange("t f j -> f t j"),
        )
        nc.sync.dma_start(
            plhs[12:18].rearrange("f (t j) -> f t j", t=T_TILES),
            pfh.ap().rearrange("t f j -> f t j"),
        )

        # ---------------- dummy gather: pre-load gpsimd ucode early -----------
        dumidx = persist.tile([128, 1], U16, tag="dumidx")
        nc.vector.memset(dumidx[:], 0)
        dumout = persist.tile([128, 32], F16, tag="dumout")
        nc.gpsimd.ap_gather(
            dumout[:].rearrange("p (i c) -> p i c", c=2),
            id16[:].rearrange("p (n c) -> p n c", c=2),
            dumidx[:].bitcast(I16),
            channels=128,
            num_elems=64,
            d=2,
            num_idxs=16,
        )

        # ---------------- per-gaussian prep ([128, 16] layout, n = 16p + j) ----
        mxy = persist.tile([128, 2 * 16], F32, tag="mxy")    # mx' | my'
        lsxy = persist.tile([128, 2 * 16], F32, tag="lsxy")  # lsx | lsy
        m3 = mjc[:].rearrange("p (j c) -> p j c", c=2)
        l3 = ljc[:].rearrange("p (j c) -> p j c", c=2)
        # deinterleave + recenter means: mx' = mx - 63.5, my' = my - yc
        nc.vector.tensor_scalar_add(mxy[:, 0:16], m3[:, :, 0], -XC)
        nc.vector.tensor_scalar(
            mxy[:, 16:32], m3[:, :, 1], yc[:], None, mybir.AluOpType.subtract
        )
        nc.vector.tensor_copy(lsxy[:, 0:16], l3[:, :, 0])
        nc.vector.tensor_copy(lsxy[:, 16:32], l3[:, :, 1])

        mx, my = mxy[:, 0:16], mxy[:, 16:32]
        lsx, lsy = lsxy[:, 0:16], lsxy[:, 16:32]

        tmp = persist.tile([128, 16 * 16], F32, tag="preptmp")

        def tt(i):
            return tmp[:, 16 * i : 16 * (i + 1)]

        cosr, sinr, ivx, ivy = tt(0), tt(1), tt(2), tt(3)
        c2, s2, sc, va, vc = tt(4), tt(5), tt(6), tt(7), tt(8)
        vb, t1, t2, t3, t4 = tt(9), tt(10), tt(11), tt(12), tt(13)
        t5, t6 = tt(14), tt(15)

        Sin = mybir.ActivationFunctionType.Sin
        Exp = mybir.ActivationFunctionType.Exp
        Copy = mybir.ActivationFunctionType.Copy
        # Scalar-engine Sin needs args in [-pi, pi]; rot is in [0, 2pi).
        # sinr := -sin(rot) = sin(rot - pi); cosr := sin(wrap(rot + pi/2)).
        negpi = persist.tile([128, 1], F32, tag="negpi")
        nc.scalar.activation(negpi[:], negpi[:], Copy, bias=float(-np.pi), scale=0.0)
        nc.scalar.activation(sinr, rot[:], Sin, bias=negpi[:])
        phi = tt(10)  # reuse t1 slot before t1 is live
        nc.vector.tensor_scalar_add(phi, rot[:], float(np.pi / 2))
        msk = tt(11)
        nc.vector.tensor_scalar(
            msk, phi, float(np.pi), float(-2.0 * np.pi),
            mybir.AluOpType.is_gt, mybir.AluOpType.mult,
        )
        nc.vector.tensor_add(phi, phi, msk)
        nc.scalar.activation(cosr, phi, Sin)
        nc.scalar.activation(ivx, lsx, Exp, scale=-2.0)
        nc.scalar.activation(ivy, lsy, Exp, scale=-2.0)
        nc.vector.tensor_mul(c2, cosr, cosr)
        nc.vector.tensor_mul(s2, sinr, sinr)
        nc.vector.tensor_mul(sc, sinr, cosr)
        # a = c2*ivx + s2*ivy ; c = s2*ivx + c2*ivy ; b = sc*(ivx-ivy)
        nc.vector.tensor_mul(t1, c2, ivx)
        nc.vector.tensor_mul(t2, s2, ivy)
        nc.vector.tensor_add(va, t1, t2)
        nc.vector.tensor_mul(t1, s2, ivx)
        nc.vector.tensor_mul(t2, c2, ivy)
        nc.vector.tensor_add(vc, t1, t2)
        # vb = sin*cos*(ivx-ivy) = (sinr*cosr)*(ivy-ivx) since sinr = -sin
        nc.vector.tensor_sub(t3, ivy, ivx)
        nc.vector.tensor_mul(vb, sc, t3)

        gbuf = persist.tile([128, 6 * 16], F32, tag="gbuf")
        # G0 = -a, G1 = -2b, G2 = -c
        nc.vector.tensor_scalar_mul(gbuf[:, 0:16], va, -1.0)
        nc.vector.tensor_scalar_mul(gbuf[:, 16:32], vb, -2.0)
        nc.vector.tensor_scalar_mul(gbuf[:, 32:48], vc, -1.0)
        # G3 = 2(a mx + b my), G4 = 2(b mx + c my)
        nc.vector.tensor_mul(t1, va, mx)      # a mx
        nc.vector.tensor_mul(t2, vb, my)      # b my
        nc.vector.tensor_add(t3, t1, t2)
        nc.vector.tensor_scalar_mul(gbuf[:, 48:64], t3, 2.0)
        nc.vector.tensor_mul(t4, vb, mx)      # b mx
        nc.vector.tensor_mul(t5, vc, my)      # c my
        nc.vector.tensor_add(t3, t4, t5)
        nc.vector.tensor_scalar_mul(gbuf[:, 64:80], t3, 2.0)
        # G5 = -(a mx^2 + 2 b mx my + c my^2)
        nc.vector.tensor_mul(t6, t1, mx)      # a mx^2
        nc.vector.tensor_mul(t3, t4, my)      # b mx my
        nc.vector.tensor_mul(t4, t5, my)      # c my^2
        nc.vector.tensor_add(t6, t6, t4)
        nc.vector.tensor_add(t3, t3, t3)
        nc.vector.tensor_add(t6, t6, t3)
        nc.vector.tensor_scalar_mul(gbuf[:, 80:96], t6, -1.0)

        # hi/lo fp16 split of G coefficients
        gh16 = persist.tile([128, 96], F16, tag="gh16")
        glo = persist.tile([128, 96], F32, tag="glo")
        gl16 = persist.tile([128, 96], F16, tag="gl16")
        nc.vector.tensor_copy(gh16[:], gbuf[:])
        nc.vector.tensor_sub(glo[:], gbuf[:], gh16[:])
        nc.vector.tensor_copy(gl16[:], glo[:])

        # PE transpose [128, 96] -> [96, 128], then scatter rows into
        # grhs [18, 2048] fp16, column order n = 128*j + p.
        xh = persist.tile([96, 128], F16, tag="xh")
        xl = persist.tile([96, 128], F16, tag="xl")
        with tc.tile_pool(name="tp", bufs=1, space="PSUM") as tp_pool:
            tph = tp_pool.tile([96, 128], F16, tag="tph")
            tpl = tp_pool.tile([96, 128], F16, tag="tpl")
            nc.tensor.transpose(tph[:], gh16[:], id16[:])
            nc.tensor.transpose(tpl[:], gl16[:], id16[:])
            nc.vector.tensor_copy(xh[:], tph[:])
            nc.vector.tensor_copy(xl[:], tpl[:])
        # partition-collapse via a small DRAM bounce (SBUF->SBUF cross-
        # partition DMA is not expressible)
        dram_pool = ctx.enter_context(tc.tile_pool(name="dram", bufs=1, space="DRAM"))
        gsc = dram_pool.tile([12, N_G], F16, tag="gsc")
        for f in range(6):
            nc.sync.dma_start(
                gsc[f : f + 1, :].rearrange("o (j p) -> (o j) p", j=16),
                xh[16 * f : 16 * (f + 1), :],
            )
            nc.sync.dma_start(
                gsc[f + 6 : f + 7, :].rearrange("o (j p) -> (o j) p", j=16),
                xl[16 * f : 16 * (f + 1), :],
            )
        grhs = persist.tile([18, N_G], F16, tag="grhs")
        nc.sync.dma_start(grhs[0:6], gsc[0:6, :])
        nc.sync.dma_start(grhs[6:12], gsc[0:6, :])
        nc.sync.dma_start(grhs[12:18], gsc[6:12, :])

        # persists across the tile loop
        Vall = persist.tile([128, T_TILES * SLOTS], F16, tag="Vall")   # t-major
        Iall = persist.tile([128, SLOTS * T_TILES], U16, tag="Iall")   # s-major
        Aall = persist.tile([128, T_TILES * SLOTS], F32, tag="Aall")   # t-major
        Iv = Iall[:].rearrange("p (s t) -> p s t", t=T_TILES)

        # ---------------- main loop over row-tiles -----------------------------
        psum_pool = ctx.enter_context(tc.tile_pool(name="ps", bufs=2, space="PSUM"))
        for t in range(T_TILES):
            S = psum_pool.tile([128, N_G], F32, tag="S")
            lt = plhs[:, PPT * t : PPT * (t + 1)]
            for q in range(4):
                nc.tensor.matmul(
                    S[:, 512 * q : 512 * (q + 1)],
                    lhsT=lt,
                    rhs=grhs[:, 512 * q : 512 * (q + 1)],
                    start=True,
                    stop=True,
                )
            S16 = work.tile([128, N_G], F16, tag="S16")
            nc.scalar.activation(S16[:], S[:], Copy)

            cand = work.tile([128, 32], F16, tag="cand")
            for q in range(4):
                nc.vector.max(cand[:, 8 * q : 8 * (q + 1)], S16[:, 512 * q : 512 * (q + 1)])

            v = Vall[:, SLOTS * t : SLOTS * t + 8]
            v2 = Vall[:, SLOTS * t + 8 : SLOTS * (t + 1)]
            nc.vector.max(v, cand[:])
            candm = work.tile([128, 32], F16, tag="candm")
            nc.vector.match_replace(candm[:], v, cand[:], NEG_SENT)
            nc.vector.max(v2, candm[:])

            nc.vector.max_index(Iv[:, 0:8, t], v, S16[:])
            nc.vector.max_index(Iv[:, 8:16, t], v2, S16[:])

        # ---------------- batched epilogue -------------------------------------
        # alphas for all tiles in one ACT instruction (junk slots -> exp -> 0)
        nc.scalar.activation(Aall[:], Vall[:], Exp, scale=0.5)

        # O = 1 - alpha
        Oall = persist.tile([128, T_TILES * SLOTS], F32, tag="Oall")
        nc.vector.tensor_scalar(
            Oall[:], Aall[:], -1.0, 1.0, mybir.AluOpType.mult, mybir.AluOpType.add
        )
        # cumprod T[k] = prod_{j<k} O[j], s-major layout [128, s(16) x t(16)]
        Tcp = persist.tile([128, SLOTS * T_TILES], F32, tag="Tcp")
        nc.vector.memset(Tcp[:, 0:16], 1.0)
        O3 = Oall[:].rearrange("p (t s) -> p s t", s=SLOTS)
        T3 = Tcp[:].rearrange("p (s t) -> p s t", t=T_TILES)
        for k in range(1, K):
            nc.vector.tensor_mul(T3[:, k : k + 1, :], T3[:, k - 1 : k, :], O3[:, k - 1 : k, :])

        Wall = persist.tile([128, T_TILES * SLOTS], F32, tag="Wall")
        W3 = Wall[:].rearrange("p (t s) -> p t s", s=SLOTS)
        A3 = Aall[:].rearrange("p (t s) -> p t s", s=SLOTS)
        T3b = Tcp[:].rearrange("p (s t) -> p t s", t=T_TILES)
        nc.vector.tensor_mul(W3[:, :, 0:K], A3[:, :, 0:K], T3b[:, :, 0:K])

        # color gather: only the K used slots (s-major => contiguous indices)
        NIDX = K * T_TILES * 16            # 2560
        G4k = persist.tile([128, NIDX * 3], F32, tag="G4k")
        nc.gpsimd.ap_gather(
            G4k[:].rearrange("p (i c) -> p i c", c=3),
            coltab[:].rearrange("p (n c) -> p n c", c=3),
            Iall[:, 0 : K * T_TILES].bitcast(I16),
            channels=128,
            num_elems=N_G,
            d=3,
            num_idxs=NIDX,
        )
        # mask out non-own-partition gathers, reduce over partition-slot axis
        nc.vector.tensor_mul(G4k[:], G4k[:], dm10[:])
        D = persist.tile([128, K * T_TILES * 3], F32, tag="D")
        nc.vector.tensor_reduce(
            D[:].rearrange("p (s t c) -> p s t c", s=K, t=T_TILES),
            G4k[:].rearrange("p (s t pp c) -> p s t c pp", s=K, t=T_TILES, pp=16),
            mybir.AxisListType.X,
            mybir.AluOpType.add,
        )
        # weights broadcast over channel, multiply, reduce over slots
        W3c = persist.tile([128, K * T_TILES * 3], F32, tag="W3c")
        Wv = Wall[:].rearrange("p (t s) -> p s t", s=SLOTS).unsqueeze(3)
        W3cv = W3c[:].rearrange("p (s t c) -> p s t c", s=K, t=T_TILES)
        for c in range(3):
            nc.vector.tensor_copy(W3cv[:, :, :, c : c + 1], Wv[:, 0:K])
        nc.vector.tensor_mul(W3c[:], W3c[:], D[:])
        outc = persist.tile([128, T_TILES * 3], F32, tag="outc")
        nc.vector.tensor_reduce(
            outc[:].rearrange("p (t c) -> p t c", t=T_TILES),
            W3c[:].rearrange("p (s t c) -> p t c s", s=K, t=T_TILES),
            mybir.AxisListType.X,
            mybir.AluOpType.add,
        )
        nc.sync.dma_start(
            out.ap().rearrange("(t j) c -> j t c", t=T_TILES),
            outc[:].rearrange("p (t c) -> p t c", t=T_TILES),
        )

    nc.compile()
    return nc


def host_constants():
    """Pixel-feature hi/lo fp16 matrices, identity, diagonal mask."""
    xs = np.arange(W, dtype=np.float64) + 0.5 - XC        # x' in [-63, 63.5]
    ys = np.arange(T_TILES, dtype=np.float64) + 0.5 - 8.0  # y' = t - 7.5
    pf = np.zeros((T_TILES, 6, PPT), np.float64)
    for t in range(T_TILES):
        y = ys[t]
        pf[t, 0] = xs * xs
        pf[t, 1] = xs * y
        pf[t, 2] = y * y
        pf[t, 3] = xs
        pf[t, 4] = y
        pf[t, 5] = 1.0
    pf_hi = pf.astype(np.float16)
    pf_lo = (pf - pf_hi.astype(np.float64)).astype(np.float16)
    ident = np.eye(128, dtype=np.float16)
    # dmask pattern [128, 48]: partition p keeps sub-slot pp = p % 16; the
    # input ships 16 repetitions ([128, 768]) so the x10 broadcast DMA uses
    # few, large descriptors.
    dm48 = np.zeros((128, 48), np.float32)
    p = np.arange(128)
    for pp in range(16):
        for c in range(3):
            dm48[p % 16 == pp, 3 * pp + c] = 1.0
    dmask = np.tile(dm48, (1, 16))
    return pf_hi, pf_lo, ident, dmask


def make_in_maps(gaussian_means, gaussian_rotations, gaussian_log_scales, gaussian_colors):
    pf_hi, pf_lo, ident, dmask = host_constants()
    # color table permuted to the grhs column order: column m = 128*j + p
    # holds gaussian n = 16*p + j.
    cols = np.ascontiguousarray(gaussian_colors, np.float32)
    ctab = np.ascontiguousarray(
        cols.reshape(128, 16, 3).transpose(1, 0, 2).reshape(N_G, 3)
    )
    base = {
        "g_means": np.ascontiguousarray(gaussian_means, np.float32),
        "g_rots": np.ascontiguousarray(gaussian_rotations, np.float32),
        "g_ls": np.ascontiguousarray(gaussian_log_scales, np.float32),
        "g_ctab": ctab,
        "pf_hi": pf_hi,
        "pf_lo": pf_lo,
        "ident": ident,
        "dmask": dmask,
    }
    in_maps = []
    for c in range(N_CORES):
        yc = np.full((128, 1), 16.0 * c + 8.0, np.float32)
        in_maps.append({**base, "ycen": yc})
    return in_maps


_NC_CACHE = {}


def _get_nc():
    if "nc" not in _NC_CACHE:
        _NC_CACHE["nc"] = build_program()
    return _NC_CACHE["nc"]


def kernel(
    gaussian_means, gaussian_rotations, gaussian_log_scales, gaussian_colors
):
    nc = _get_nc()
    in_maps = make_in_maps(
        gaussian_means, gaussian_rotations, gaussian_log_scales, gaussian_colors
    )
    res = run_bass_kernel_spmd(nc, in_maps, list(range(N_CORES)))
    rows = [res.results[c]["out"].reshape(ROWS_PER_CORE, W, 3) for c in range(N_CORES)]
    return np.concatenate(rows, axis=0)


if __name__ == "__main__":
    ins = {
        "gaussian_means": np.random.rand(N_G, 2).astype(np.float32) * [W, H],
        "gaussian_rotations": np.random.rand(N_G).astype(np.float32) * 2 * np.pi,
        "gaussian_log_scales": (np.random.randn(N_G, 2) * 0.3 + np.log(3)).astype(
            np.float32
        ),
        "gaussian_colors": np.random.rand(N_G, 3).astype(np.float32),
    }
    img = kernel(**ins)
    print(img.shape, img.dtype, img.mean())


# revision 19
# speedup vs baseline: 1.0196x; 1.0196x over previous
"""Gaussian splatting renderer on 8 Trainium2 NeuronCores.

v2 — optimized pipeline (baseline was 375us):
  - S[p, n] = -Mahalanobis^2 via a split-fp16 rank-18 matmul (hi/lo
    Dekker split of both operands, coords recentered per core so fp16
    magnitudes stay small).  fp16 streams 1 col/cycle vs ~3 for fp32.
  - ACT converts PSUM fp32 -> SBUF fp16 score array (graded float
    precision is ideal: top alphas have |S| near 0).
  - DVE top-k on fp16 at 2x rate: max8 per 512-chunk, merge with
    max8/match_replace/max8 (ranks 1..16), find_index8 for ids.
  - exp batched into one ACT instruction at the end (no table thrash).
  - per-gaussian features computed on device in [128,16] layout, then
    PE-transposed (no DRAM round trip); gather color table fed
    host-permuted so transposed column order needs no index fixup.
  - dummy early ap_gather pre-loads the gpsimd ucode so its expensive
    drain overlaps the main loop instead of serializing the epilogue.
  - color gather only fetches the K=10 used slots (s-major index
    layout), diagonal-mask + reduce as in baseline but 37% smaller.
"""

import os
import sys

sys.path.insert(0, "/opt/trn_rl_repo")

import numpy as np
from contextlib import ExitStack

from concourse import bacc, bass, mybir, tile
from concourse.bass_utils import run_bass_kernel_spmd

H, W, K = 128, 128, 10
N_G = 2048
N_CORES = 8
ROWS_PER_CORE = H // N_CORES          # 16
T_TILES = ROWS_PER_CORE               # one image row per tile
PPT = 128                             # pixels per tile (one row)
SLOTS = 16                            # padded top-k slots (ranks 1..16)
F32 = mybir.dt.float32
F16 = mybir.dt.float16
U16 = mybir.dt.uint16
I16 = mybir.dt.int16

NEG_SENT = -60000.0                   # fp16-representable sentinel
XC = 63.5                             # x recenter offset


def build_program():
    nc = bacc.Bacc(
        "TRN2",
        target_bir_lowering=False,
        debug=False,
        num_devices=N_CORES,
    )

    means = nc.dram_tensor("g_means", [N_G, 2], F32, kind="ExternalInput")
    rots = nc.dram_tensor("g_rots", [N_G], F32, kind="ExternalInput")
    lss = nc.dram_tensor("g_ls", [N_G, 2], F32, kind="ExternalInput")
    ctab = nc.dram_tensor("g_ctab", [N_G, 3], F32, kind="ExternalInput")
    pfh = nc.dram_tensor("pf_hi", [T_TILES, 6, PPT], F16, kind="ExternalInput")
    pfl = nc.dram_tensor("pf_lo", [T_TILES, 6, PPT], F16, kind="ExternalInput")
    ident = nc.dram_tensor("ident", [128, 128], F16, kind="ExternalInput")
    dmask = nc.dram_tensor("dmask", [128, 768], F32, kind="ExternalInput")
    ycen = nc.dram_tensor("ycen", [128, 1], F32, kind="ExternalInput")
    out = nc.dram_tensor("out", [ROWS_PER_CORE * W, 3], F32, kind="ExternalOutput")

    with tile.TileContext(nc) as tc, ExitStack() as ctx:
        persist = ctx.enter_context(tc.tile_pool(name="persist", bufs=1))
        work = ctx.enter_context(tc.tile_pool(name="work", bufs=3))

        # ---------------- input DMAs (all contiguous, sync queue) -------------
        mjc = persist.tile([128, 32], F32, tag="mjc")      # means (j c) interleaved
        rot = persist.tile([128, 16], F32, tag="rot")
        ljc = persist.tile([128, 32], F32, tag="ljc")      # log scales (j c)
        id16 = persist.tile([128, 128], F16, tag="id16")
        yc = persist.tile([128, 1], F32, tag="yc")
        # prep-critical loads first (rot gates the longest ACT chain)
        nc.sync.dma_start(rot[:], rots.ap().rearrange("(p j) -> p j", p=128))
        nc.sync.dma_start(mjc[:], means.ap().rearrange("(p j) c -> p (j c)", p=128))
        nc.sync.dma_start(ljc[:], lss.ap().rearrange("(p j) c -> p (j c)", p=128))
        nc.sync.dma_start(yc[:], ycen.ap())
        nc.sync.dma_start(id16[:], ident.ap())

        # pixel features lhsT [18, 16*128] fp16: rows 0-5 hi, 6-11 lo, 12-17 hi
        plhs = persist.tile([18, T_TILES * PPT], F16, tag="plhs")
        nc.sync.dma_start(
            plhs[0:6].rearrange("f (t j) -> f t j", t=T_TILES),
            pfh.ap().rearrange("t f j -> f t j"),
        )
        nc.sync.dma_start(
            plhs[6:12].rearrange("f (t j) -> f t j", t=T_TILES),
            pfl.ap().rearrange("t f j -> f t j"),
        )
        nc.sync.dma_start(
            plhs[12:18].rearrange("f (t j) -> f t j", t=T_TILES),
            pfh.ap().rearrange("t f j -> f t j"),
        )

        # epilogue-only loads (big broadcasts) issued after the critical ones
        coltab = persist.tile([128, N_G * 3], F32, tag="coltab")
        nc.sync.dma_start(
            coltab[:],
            ctab.ap().rearrange("n c -> (n c)").unsqueeze(0).broadcast_to([128, N_G * 3]),
        )
        dm10 = persist.tile([128, K * T_TILES * 48], F32, tag="dm10")
        nc.sync.dma_start(
            dm10[:].rearrange("p (u q) -> p u q", q=768),
            dmask.ap().unsqueeze(1).broadcast_to([128, K, 768]),
        )

        # ---------------- dummy gather: pre-load gpsimd ucode early -----------
        dumidx = persist.tile([128, 1], U16, tag="dumidx")
        nc.vector.memset(dumidx[:], 0)
        dumout = persist.tile([128, 32], F16, tag="dumout")
        nc.gpsimd.ap_gather(
            dumout[:].rearrange("p (i c) -> p i c", c=2),
            id16[:].rearrange("p (n c) -> p n c", c=2),
            dumidx[:].bitcast(I16),
            channels=128,
            num_elems=64,
            d=2,
            num_idxs=16,
        )

        # ---------------- per-gaussian prep ([128, 16] layout, n = 16p + j) ----
        mxy = persist.tile([128, 2 * 16], F32, tag="mxy")    # mx' | my'
        lsxy = persist.tile([128, 2 * 16], F32, tag="lsxy")  # lsx | lsy
        m3 = mjc[:].rearrange("p (j c) -> p j c", c=2)
        l3 = ljc[:].rearrange("p (j c) -> p j c", c=2)
        # deinterleave + recenter means: mx' = mx - 63.5, my' = my - yc
        nc.vector.tensor_scalar_add(mxy[:, 0:16], m3[:, :, 0], -XC)
        nc.vector.tensor_scalar(
            mxy[:, 16:32], m3[:, :, 1], yc[:], None, mybir.AluOpType.subtract
        )
        nc.vector.tensor_copy(lsxy[:, 0:16], l3[:, :, 0])
        nc.vector.tensor_copy(lsxy[:, 16:32], l3[:, :, 1])

        mx, my = mxy[:, 0:16], mxy[:, 16:32]
        lsx, lsy = lsxy[:, 0:16], lsxy[:, 16:32]

        tmp = persist.tile([128, 16 * 16], F32, tag="preptmp")

        def tt(i):
            return tmp[:, 16 * i : 16 * (i + 1)]

        cosr, sinr, ivx, ivy = tt(0), tt(1), tt(2), tt(3)
        c2, s2, sc, va, vc = tt(4), tt(5), tt(6), tt(7), tt(8)
        vb, t1, t2, t3, t4 = tt(9), tt(10), tt(11), tt(12), tt(13)
        t5, t6 = tt(14), tt(15)

        Sin = mybir.ActivationFunctionType.Sin
        Exp = mybir.ActivationFunctionType.Exp
        Copy = mybir.ActivationFunctionType.Copy
        # Scalar-engine Sin needs args in [-pi, pi]; rot is in [0, 2pi).
        # sinr := -sin(rot) = sin(rot - pi); cosr := sin(wrap(rot + pi/2)).
        negpi = persist.tile([128, 1], F32, tag="negpi")
        nc.scalar.activation(negpi[:], negpi[:], Copy, bias=float(-np.pi), scale=0.0)
        nc.scalar.activation(sinr, rot[:], Sin, bias=negpi[:])
        phi = tt(10)  # reuse t1 slot before t1 is live
        nc.vector.tensor_scalar_add(phi, rot[:], float(np.pi / 2))
        msk = tt(11)
        nc.vector.tensor_scalar(
            msk, phi, float(np.pi), float(-2.0 * np.pi),
            mybir.AluOpType.is_gt, mybir.AluOpType.mult,
        )
        nc.vector.tensor_add(phi, phi, msk)
        nc.scalar.activation(cosr, phi, Sin)
        nc.scalar.activation(ivx, lsx, Exp, scale=-2.0)
        nc.scalar.activation(ivy, lsy, Exp, scale=-2.0)
        nc.vector.tensor_mul(c2, cosr, cosr)
        nc.vector.tensor_mul(s2, sinr, sinr)
        nc.vector.tensor_mul(sc, sinr, cosr)
        # a = c2*ivx + s2*ivy ; c = s2*ivx + c2*ivy ; b = sc*(ivx-ivy)
        nc.vector.tensor_mul(t1, c2, ivx)
        nc.vector.tensor_mul(t2, s2, ivy)
        nc.vector.tensor_add(va, t1, t2)
        nc.vector.tensor_mul(t1, s2, ivx)
        nc.vector.tensor_mul(t2, c2, ivy)
        nc.vector.tensor_add(vc, t1, t2)
        # vb = sin*cos*(ivx-ivy) = (sinr*cosr)*(ivy-ivx) since sinr = -sin
        nc.vector.tensor_sub(t3, ivy, ivx)
        nc.vector.tensor_mul(vb, sc, t3)

        gbuf = persist.tile([128, 6 * 16], F32, tag="gbuf")
        # G0 = -a, G1 = -2b, G2 = -c
        nc.vector.tensor_scalar_mul(gbuf[:, 0:16], va, -1.0)
        nc.vector.tensor_scalar_mul(gbuf[:, 16:32], vb, -2.0)
        nc.vector.tensor_scalar_mul(gbuf[:, 32:48], vc, -1.0)
        # G3 = 2(a mx + b my), G4 = 2(b mx + c my)
        nc.vector.tensor_mul(t1, va, mx)      # a mx
        nc.vector.tensor_mul(t2, vb, my)      # b my
        nc.vector.tensor_add(t3, t1, t2)
        nc.vector.tensor_scalar_mul(gbuf[:, 48:64], t3, 2.0)
        nc.vector.tensor_mul(t4, vb, mx)      # b mx
        nc.vector.tensor_mul(t5, vc, my)      # c my
        nc.vector.tensor_add(t3, t4, t5)
        nc.vector.tensor_scalar_mul(gbuf[:, 64:80], t3, 2.0)
        # G5 = -(a mx^2 + 2 b mx my + c my^2)
        nc.vector.tensor_mul(t6, t1, mx)      # a mx^2
        nc.vector.tensor_mul(t3, t4, my)      # b mx my
        nc.vector.tensor_mul(t4, t5, my)      # c my^2
        nc.vector.tensor_add(t6, t6, t4)
        nc.vector.tensor_add(t3, t3, t3)
        nc.vector.tensor_add(t6, t6, t3)
        nc.vector.tensor_scalar_mul(gbuf[:, 80:96], t6, -1.0)

        # hi/lo fp16 split of G coefficients
        gh16 = persist.tile([128, 96], F16, tag="gh16")
        glo = persist.tile([128, 96], F32, tag="glo")
        gl16 = persist.tile([128, 96], F16, tag="gl16")
        nc.vector.tensor_copy(gh16[:], gbuf[:])
        nc.vector.tensor_sub(glo[:], gbuf[:], gh16[:])
        nc.vector.tensor_copy(gl16[:], glo[:])

        # PE transpose [128, 96] -> [96, 128], then scatter rows into
        # grhs [18, 2048] fp16, column order n = 128*j + p.
        xh = persist.tile([96, 128], F16, tag="xh")
        xl = persist.tile([96, 128], F16, tag="xl")
        with tc.tile_pool(name="tp", bufs=1, space="PSUM") as tp_pool:
            tph = tp_pool.tile([96, 128], F16, tag="tph")
            tpl = tp_pool.tile([96, 128], F16, tag="tpl")
            nc.tensor.transpose(tph[:], gh16[:], id16[:])
            nc.tensor.transpose(tpl[:], gl16[:], id16[:])
            nc.vector.tensor_copy(xh[:], tph[:])
            nc.vector.tensor_copy(xl[:], tpl[:])
        # partition-collapse via a small DRAM bounce (SBUF->SBUF cross-
        # partition DMA is not expressible)
        dram_pool = ctx.enter_context(tc.tile_pool(name="dram", bufs=1, space="DRAM"))
        gsc = dram_pool.tile([12, N_G], F16, tag="gsc")
        for f in range(6):
            nc.sync.dma_start(
                gsc[f : f + 1, :].rearrange("o (j p) -> (o j) p", j=16),
                xh[16 * f : 16 * (f + 1), :],
            )
            nc.sync.dma_start(
                gsc[f + 6 : f + 7, :].rearrange("o (j p) -> (o j) p", j=16),
                xl[16 * f : 16 * (f + 1), :],
            )
        grhs = persist.tile([18, N_G], F16, tag="grhs")
        nc.sync.dma_start(grhs[0:6], gsc[0:6, :])
        nc.sync.dma_start(grhs[6:12], gsc[0:6, :])
        nc.sync.dma_start(grhs[12:18], gsc[6:12, :])

        # persists across the tile loop
        Vall = persist.tile([128, T_TILES * SLOTS], F16, tag="Vall")   # t-major
        Iall = persist.tile([128, SLOTS * T_TILES], U16, tag="Iall")   # s-major
        Aall = persist.tile([128, T_TILES * SLOTS], F32, tag="Aall")   # t-major
        Iv = Iall[:].rearrange("p (s t) -> p s t", t=T_TILES)

        # ---------------- main loop over row-tiles -----------------------------
        psum_pool = ctx.enter_context(tc.tile_pool(name="ps", bufs=2, space="PSUM"))
        for t in range(T_TILES):
            S = psum_pool.tile([128, N_G], F32, tag="S")
            lt = plhs[:, PPT * t : PPT * (t + 1)]
            for q in range(4):
                nc.tensor.matmul(
                    S[:, 512 * q : 512 * (q + 1)],
                    lhsT=lt,
                    rhs=grhs[:, 512 * q : 512 * (q + 1)],
                    start=True,
                    stop=True,
                )
            S16 = work.tile([128, N_G], F16, tag="S16")
            nc.scalar.activation(S16[:], S[:], Copy)

            cand = work.tile([128, 16], F16, tag="cand")
            for q in range(2):
                nc.vector.max(cand[:, 8 * q : 8 * (q + 1)], S16[:, 1024 * q : 1024 * (q + 1)])

            v = Vall[:, SLOTS * t : SLOTS * t + 8]
            v2 = Vall[:, SLOTS * t + 8 : SLOTS * (t + 1)]
            nc.vector.max(v, cand[:])
            candm = work.tile([128, 16], F16, tag="candm")
            nc.vector.match_replace(candm[:], v, cand[:], NEG_SENT)
            nc.vector.max(v2, candm[:])

            nc.vector.max_index(Iv[:, 0:8, t], v, S16[:])
            nc.vector.max_index(Iv[:, 8:16, t], v2, S16[:])

        # ---------------- batched epilogue -------------------------------------
        # alphas for all tiles in one ACT instruction (junk slots -> exp -> 0)
        nc.scalar.activation(Aall[:], Vall[:], Exp, scale=0.5)

        # O = 1 - alpha
        Oall = persist.tile([128, T_TILES * SLOTS], F32, tag="Oall")
        nc.vector.tensor_scalar(
            Oall[:], Aall[:], -1.0, 1.0, mybir.AluOpType.mult, mybir.AluOpType.add
        )
        # cumprod T[k] = prod_{j<k} O[j], s-major layout [128, s(16) x t(16)]
        Tcp = persist.tile([128, SLOTS * T_TILES], F32, tag="Tcp")
        nc.vector.memset(Tcp[:, 0:16], 1.0)
        O3 = Oall[:].rearrange("p (t s) -> p s t", s=SLOTS)
        T3 = Tcp[:].rearrange("p (s t) -> p s t", t=T_TILES)
        for k in range(1, K):
            nc.vector.tensor_mul(T3[:, k : k + 1, :], T3[:, k - 1 : k, :], O3[:, k - 1 : k, :])

        Wall = persist.tile([128, T_TILES * SLOTS], F32, tag="Wall")
        W3 = Wall[:].rearrange("p (t s) -> p t s", s=SLOTS)
        A3 = Aall[:].rearrange("p (t s) -> p t s", s=SLOTS)
        T3b = Tcp[:].rearrange("p (s t) -> p t s", t=T_TILES)
        nc.vector.tensor_mul(W3[:, :, 0:K], A3[:, :, 0:K], T3b[:, :, 0:K])

        # late dummy gather: absorbs the expensive pre-custom-op drain while
        # depending on the same Iall data, so the real gather's drain is cheap
        dum2 = persist.tile([128, 192], F32, tag="dum2")
        nc.gpsimd.ap_gather(
            dum2[:].rearrange("p (i c) -> p i c", c=3),
            coltab[:].rearrange("p (n c) -> p n c", c=3),
            Iall[:, 0:4].bitcast(I16),
            channels=128,
            num_elems=N_G,
            d=3,
            num_idxs=64,
        )

        # color gather: only the K used slots (s-major => contiguous indices)
        NIDX = K * T_TILES * 16            # 2560
        G4k = persist.tile([128, NIDX * 3], F32, tag="G4k")
        nc.gpsimd.ap_gather(
            G4k[:].rearrange("p (i c) -> p i c", c=3),
            coltab[:].rearrange("p (n c) -> p n c", c=3),
            Iall[:, 0 : K * T_TILES].bitcast(I16),
            channels=128,
            num_elems=N_G,
            d=3,
            num_idxs=NIDX,
        )
        # mask out non-own-partition gathers, reduce over partition-slot axis
        nc.vector.tensor_mul(G4k[:], G4k[:], dm10[:])
        D = persist.tile([128, K * T_TILES * 3], F32, tag="D")
        nc.vector.tensor_reduce(
            D[:].rearrange("p (s t c) -> p s t c", s=K, t=T_TILES),
            G4k[:].rearrange("p (s t pp c) -> p s t c pp", s=K, t=T_TILES, pp=16),
            mybir.AxisListType.X,
            mybir.AluOpType.add,
        )
        # weights broadcast over channel, multiply, reduce over slots
        W3c = persist.tile([128, K * T_TILES * 3], F32, tag="W3c")
        Wv = Wall[:].rearrange("p (t s) -> p s t", s=SLOTS).unsqueeze(3)
        W3cv = W3c[:].rearrange("p (s t c) -> p s t c", s=K, t=T_TILES)
        for c in range(3):
            nc.vector.tensor_copy(W3cv[:, :, :, c : c + 1], Wv[:, 0:K])
        nc.vector.tensor_mul(W3c[:], W3c[:], D[:])
        outc = persist.tile([128, T_TILES * 3], F32, tag="outc")
        nc.vector.tensor_reduce(
            outc[:].rearrange("p (t c) -> p t c", t=T_TILES),
            W3c[:].rearrange("p (s t c) -> p t c s", s=K, t=T_TILES),
            mybir.AxisListType.X,
            mybir.AluOpType.add,
        )
        nc.sync.dma_start(
            out.ap().rearrange("(t j) c -> j t c", t=T_TILES),
            outc[:].rearrange("p (t c) -> p t c", t=T_TILES),
        )

    nc.compile()
    return nc


def host_constants():
    """Pixel-feature hi/lo fp16 matrices, identity, diagonal mask."""
    xs = np.arange(W, dtype=np.float64) + 0.5 - XC        # x' in [-63, 63.5]
    ys = np.arange(T_TILES, dtype=np.float64) + 0.5 - 8.0  # y' = t - 7.5
    pf = np.zeros((T_TILES, 6, PPT), np.float64)
    for t in range(T_TILES):
        y = ys[t]
        pf[t, 0] = xs * xs
        pf[t, 1] = xs * y
        pf[t, 2] = y * y
        pf[t, 3] = xs
        pf[t, 4] = y
        pf[t, 5] = 1.0
    pf_hi = pf.astype(np.float16)
    pf_lo = (pf - pf_hi.astype(np.float64)).astype(np.float16)
    ident = np.eye(128, dtype=np.float16)
    # dmask pattern [128, 48]: partition p keeps sub-slot pp = p % 16; the
    # input ships 16 repetitions ([128, 768]) so the x10 broadcast DMA uses
    # few, large descriptors.
    dm48 = np.zeros((128, 48), np.float32)
    p = np.arange(128)
    for pp in range(16):
        for c in range(3):
            dm48[p % 16 == pp, 3 * pp + c] = 1.0
    dmask = np.tile(dm48, (1, 16))
    return pf_hi, pf_lo, ident, dmask


def make_in_maps(gaussian_means, gaussian_rotations, gaussian_log_scales, gaussian_colors):
    pf_hi, pf_lo, ident, dmask = host_constants()
    # color table permuted to the grhs column order: column m = 128*j + p
    # holds gaussian n = 16*p + j.
    cols = np.ascontiguousarray(gaussian_colors, np.float32)
    ctab = np.ascontiguousarray(
        cols.reshape(128, 16, 3).transpose(1, 0, 2).reshape(N_G, 3)
    )
    base = {
        "g_means": np.ascontiguousarray(gaussian_means, np.float32),
        "g_rots": np.ascontiguousarray(gaussian_rotations, np.float32),
        "g_ls": np.ascontiguousarray(gaussian_log_scales, np.float32),
        "g_ctab": ctab,
        "pf_hi": pf_hi,
        "pf_lo": pf_lo,
        "ident": ident,
        "dmask": dmask,
    }
    in_maps = []
    for c in range(N_CORES):
        yc = np.full((128, 1), 16.0 * c + 8.0, np.float32)
        in_maps.append({**base, "ycen": yc})
    return in_maps


_NC_CACHE = {}


def _get_nc():
    if "nc" not in _NC_CACHE:
        _NC_CACHE["nc"] = build_program()
    return _NC_CACHE["nc"]


def kernel(
    gaussian_means, gaussian_rotations, gaussian_log_scales, gaussian_colors
):
    nc = _get_nc()
    in_maps = make_in_maps(
        gaussian_means, gaussian_rotations, gaussian_log_scales, gaussian_colors
    )
    res = run_bass_kernel_spmd(nc, in_maps, list(range(N_CORES)))
    rows = [res.results[c]["out"].reshape(ROWS_PER_CORE, W, 3) for c in range(N_CORES)]
    return np.concatenate(rows, axis=0)


if __name__ == "__main__":
    ins = {
        "gaussian_means": np.random.rand(N_G, 2).astype(np.float32) * [W, H],
        "gaussian_rotations": np.random.rand(N_G).astype(np.float32) * 2 * np.pi,
        "gaussian_log_scales": (np.random.randn(N_G, 2) * 0.3 + np.log(3)).astype(
            np.float32
        ),
        "gaussian_colors": np.random.rand(N_G, 3).astype(np.float32),
    }
    img = kernel(**ins)
    print(img.shape, img.dtype, img.mean())
